# revision 36
# baseline (speedup 1.0000x reference)
"""Encoder-decoder attention kernel for Trainium2, 8 NeuronCores.

Sharding: batch (B=8) data-parallel, one batch element per core; weights
replicated. Per core (S=Sq=Sk=1024, H=1024, NH=16, D=64):

  phase A: transpose X_dec, X_enc via PE -> x_dec_t/x_enc_t [h,s] f32r
           (+ fp16 copy of X_enc^T on ACT for the V projection)
  per pair p (heads 2p, 2p+1), prep woven into the qt loop as half-sized
  chunks so the PE never idles (HAM stays at K=8/8):
    Q^T/K^T proj: fp32r matmuls, w tiles raw-bit f32r DMA, accumulated
      in [128,512] PSUM halves; Q written as zero-padded per-head tiles
      (head h data in its own partition rows, zeros elsewhere) so a
      K=128 fp32r scores matmul against the k pair tile contracts only
      head h; pad halves are zeroed once and the tiles slot-rotate
    V proj: fp16 (x_enc_h stationary, cached fp16 w tiles), [128,512]
      PSUM halves -> v2 [k,nd] fp16
    scores: per qt, 4 fp32r K=128 matmuls (2 heads x 2 k-halves) into
      two [128,1024] PSUM tiles; per-half DVE negated-max reduces run
      under the matmuls, combined via min
    softmax: ACT exp(bias=-max, accum_out=rowsum) -> DVE reciprocal ->
      fp16 normalize split DVE(head0)/GPSIMD(head1) -> per-qt hw DMA
      transpose ([128,1024]->[128,8,128]) on Sync
    O^T = V^T P^T in fp16, ev/od heads col-tiled, emitted one block lag
  phase D: out = concat @ W_out^T + b_out (fp16 matmuls, DVE bias add)

Precision: fp32r (~1.5e-4 mult rounding) through Q/K/scores gives score
abs err ~0.06 (scores ~N(0,341)); final rel err ~2e-3 vs the 2e-2 gate.
P and V run in fp16. 1/sqrt(D)=1/8 folded into W_query on the host.
"""
import sys

sys.path.insert(0, "/opt/trn_rl_repo")

import numpy as np

B = 8
S = 1024   # Sq == Sk
H = 1024
NH = 16
D = 64
P = 128
HT = H // P    # 8 h-tiles
ST = S // P    # 8 s-tiles == k-tiles
NP = NH // 2   # 8 head pairs
QB = 256       # q-block width for the P@V moving dim
NB = S // QB   # 4 q-blocks
QTB = QB // P  # 2 q-tiles per block


def build():
    import concourse.mybir as mybir
    import concourse.tile as tile
    from concourse import bacc
    from concourse.masks import make_identity

    f32 = mybir.dt.float32
    f32r = mybir.dt.float32r
    f16 = mybir.dt.float16
    AX = mybir.AxisListType.X
    OP = mybir.AluOpType
    AF = mybir.ActivationFunctionType

    nc = bacc.Bacc(trn_type="TRN2", target_bir_lowering=False, debug=False)

    xd_d = nc.dram_tensor("xd", [S, H], f32, kind="ExternalInput").ap()
    xe_d = nc.dram_tensor("xe", [S, H], f32, kind="ExternalInput").ap()
    # f32r raw bits == f32 bits (verified on hw); lets plain DMA feed
    # fp32r matmuls directly
    wqt_d = nc.dram_tensor("wqt", [H, H], f32r, kind="ExternalInput").ap()  # [h, nd] (pre-scaled 1/8)
    wkt_d = nc.dram_tensor("wkt", [H, H], f32r, kind="ExternalInput").ap()  # [h, nd]
    wvt_d = nc.dram_tensor("wvt", [H, H], f32, kind="ExternalInput").ap()   # [h, nd]
    wot_d = nc.dram_tensor("wot", [H, H], f32, kind="ExternalInput").ap()   # [nd, h_out]
    bias_d = nc.dram_tensor("bias", [P, H], f32, kind="ExternalInput").ap()
    out_d = nc.dram_tensor("out", [S, H], f32, kind="ExternalOutput").ap()

    from contextlib import ExitStack
    with tile.TileContext(nc) as tc:
        with ExitStack() as ctx:
            big = ctx.enter_context(tc.tile_pool(name="big", bufs=16))
            qtp = ctx.enter_context(tc.tile_pool(name="qt", bufs=1))
            ksp = ctx.enter_context(tc.tile_pool(name="ks", bufs=2))
            vpp = ctx.enter_context(tc.tile_pool(name="vp", bufs=2))
            ccp = ctx.enter_context(tc.tile_pool(name="cc", bufs=NP))
            xehp = ctx.enter_context(tc.tile_pool(name="xeh", bufs=8))
            xinp = ctx.enter_context(tc.tile_pool(name="xin", bufs=1))
            pep = ctx.enter_context(tc.tile_pool(name="pe", bufs=3))
            ptp = ctx.enter_context(tc.tile_pool(name="pt", bufs=4))
            wtp = ctx.enter_context(tc.tile_pool(name="wt", bufs=2))
            wvp = ctx.enter_context(tc.tile_pool(name="wv", bufs=1))
            wvcp = ctx.enter_context(tc.tile_pool(name="wvc", bufs=8))
            wop = ctx.enter_context(tc.tile_pool(name="wo", bufs=2))
            worp = ctx.enter_context(tc.tile_pool(name="wor", bufs=4))
            osbp = ctx.enter_context(tc.tile_pool(name="osb", bufs=1))
            constp = ctx.enter_context(tc.tile_pool(name="const", bufs=1))
            statp = ctx.enter_context(tc.tile_pool(name="stat", bufs=24))
            psp = ctx.enter_context(tc.tile_pool(name="ps", bufs=2, space="PSUM"))
            psSp = ctx.enter_context(tc.tile_pool(name="psS", bufs=3, space="PSUM"))

            def pstile():
                return psp.tile([P, 512], f32, tag="ps", name="ps")

            def pstileS():
                return psSp.tile([P, S], f32, tag="psS", name="psS")

            def stat():
                return statp.tile([P, 1], f32, tag="stat", name="stat")

            # ---- constants ----
            ident = constp.tile([P, P], f32)
            make_identity(nc, ident[:])
            bias_sb = constp.tile([P, H], f32)
            nc.scalar.dma_start(bias_sb[:], bias_d)
            # warmup transpose absorbs the gpsimd(identity) dep on PE
            warm = pstile()
            nc.tensor.transpose(warm[:, 0:P], ident[:], ident[:])
            zeros = constp.tile([P, 512], f32)
            nc.vector.memset(zeros[:], 0.0)

            # persistent zero-padded Q tiles: 2 slots x (head0, head1);
            # pad halves zeroed once, data halves rewritten per pair
            q_slots = []
            for sl in range(2):
                qp0 = qtp.tile([P, S], f32r, name=f"qp0_{sl}")
                qp1 = qtp.tile([P, S], f32r, name=f"qp1_{sl}")
                for nn in range(2):
                    ns = slice(nn * 512, (nn + 1) * 512)
                    nc.vector.tensor_copy(qp0[64:128, ns], zeros[64:128, :])
                    nc.vector.tensor_copy(qp1[0:64, ns], zeros[0:64, :])
                q_slots.append((qp0, qp1))

            # ---- block weight loads (one DMA per pair-projection) ----
            q_wt = {}
            k_wt = {}

            def load_wt(dst_map, dram, p, tag):
                wt = wtp.tile([P, HT, P], f32r, tag=tag, name=tag)
                nc.gpsimd.dma_start(
                    wt[:],
                    dram[:, p * P:(p + 1) * P]
                    .rearrange("(j q) c -> q j c", j=HT))
                dst_map[p] = wt

            # ---- prep chunks (emitted JIT, ~half-chunk per qt slot) ----
            q_t = {}
            k_s_next = [None]
            v2_next = [None]

            def q_chunks(p):
                def half(nn):
                    ns = slice(nn * 512, (nn + 1) * 512)
                    psh = pstile()
                    w = q_wt[p]
                    for j in range(HT):
                        nc.tensor.matmul(
                            psh[:], w[:, j, :], x_dec_t[j][:, ns],
                            start=(j == 0), stop=(j == HT - 1))
                    qp0, qp1 = q_slots[p % 2]
                    nc.scalar.activation(qp0[0:64, ns], psh[0:64, :],
                                         AF.Copy)
                    nc.scalar.activation(qp1[64:128, ns], psh[64:128, :],
                                         AF.Copy)

                def c1():
                    q_t[p] = q_slots[p % 2]
                    half(0)

                def c2():
                    half(1)

                return [c1, c2]

            def k_chunks(p):
                box = {}

                def half(nn):
                    ns = slice(nn * 512, (nn + 1) * 512)
                    psh = pstile()
                    w = k_wt[p]
                    for j in range(HT):
                        nc.tensor.matmul(
                            psh[:], w[:, j, :], x_enc_t[j][:, ns],
                            start=(j == 0), stop=(j == HT - 1))
                    nc.scalar.activation(box['k'][:, ns], psh[:], AF.Copy)

                def c1():
                    box['k'] = ksp.tile([P, S], f32r, tag="ks", name="ksb")
                    half(0)

                def c2():
                    half(1)
                    k_s_next[0] = box['k']

                return [c1, c2]

            def v_chunks(p):
                # group of 4 pairs (nd cols p*128..(p+4)*128); 8 pieces,
                # one k-tile each; fp16 w tiles cached for the group
                box = {'wh': None}

                def piece(kt_i):
                    def ci():
                        if box['wh'] is None:
                            box['wh'] = []
                            for j in range(HT):
                                wvs = wvp.tile([P, 512], f32, tag="wv",
                                               name="wv")
                                nc.gpsimd.dma_start(
                                    wvs[:],
                                    wvt_d[j * P:(j + 1) * P,
                                          p * P:(p + 4) * P])
                                wh = wvcp.tile([P, 512], f16, tag="wvh",
                                               name="wvh")
                                nc.scalar.activation(wh[:], wvs[:], AF.Copy)
                                box['wh'].append(wh)
                        if kt_i == 0:
                            box['v2'] = vpp.tile([P, ST, 512], f16,
                                                 tag="vp", name="v2")
                            v2_next[0] = box['v2']
                        psh = pstile()
                        for j in range(HT):
                            nc.tensor.matmul(
                                psh[:],
                                x_enc_h[j][:, kt_i * P:(kt_i + 1) * P],
                                box['wh'][j][:],
                                start=(j == 0), stop=(j == HT - 1))
                        nc.scalar.activation(box['v2'][:, kt_i, :], psh[:],
                                             AF.Copy)
                    return ci

                return [piece(i) for i in range(ST)]

            # ---- phase A + prologue, interleaved ----
            # dec transposes first; then enc transposes with the prologue
            # chunks (Q0/K0 halves, V group-A pieces) woven between s-tiles
            # so the PE runs real matmuls early (transposes don't count as
            # HAM activity, so a pure-transpose phase stays at 1.2 GHz)
            x_dec_t = [big.tile([P, S], f32r, tag="big", name="xdt")
                       for _ in range(HT)]
            x_enc_t = [big.tile([P, S], f32r, tag="big", name="xet")
                       for _ in range(HT)]
            x_enc_h = [xehp.tile([P, S], f16, tag="xeh", name="xeh")
                       for _ in range(HT)]
            load_wt(q_wt, wqt_d, 0, "qw")
            load_wt(k_wt, wkt_d, 0, "kw")
            load_wt(q_wt, wqt_d, 1, "qw")
            load_wt(k_wt, wkt_d, 1, "kw")
            qc0 = q_chunks(0)
            kc0 = k_chunks(0)
            vA = v_chunks(0)

            def phase_a_tile(lst, src, i):
                # PSUM->SBUF drains split across DVE and ACT per group so
                # neither engine gates the transpose stream
                xin = xinp.tile([P, H], f32, tag="xin")
                nc.sync.dma_start(xin[:], src[i * P:(i + 1) * P, :])
                for g in range(2):
                    pst = pstile()
                    for t in range(4):
                        j = g * 4 + t
                        nc.tensor.transpose(
                            pst[:, t * P:(t + 1) * P],
                            xin[:, j * P:(j + 1) * P], ident[:])
                    for t in range(4):
                        j = g * 4 + t
                        sl = pst[:, t * P:(t + 1) * P]
                        dst = lst[j][:, i * P:(i + 1) * P]
                        if g == 0:
                            nc.vector.tensor_copy(dst, sl)
                        else:
                            nc.scalar.activation(dst, sl, AF.Copy)
                        if lst is x_enc_t:
                            hdst = x_enc_h[j][:, i * P:(i + 1) * P]
                            if g == 0:
                                nc.scalar.activation(hdst, sl, AF.Copy)
                            else:
                                nc.vector.tensor_copy(hdst, sl)

            with nc.named_scope("phaseA"):
                for i in range(ST):
                    phase_a_tile(x_dec_t, xd_d, i)
                # chunks lag one s-tile so their inputs (the previous
                # tile's PSUM->SBUF copies) are already drained
                post = {0: [], 1: [vA[0]], 2: [vA[1], qc0[0]],
                        3: [vA[2]], 4: [vA[3], qc0[1]],
                        5: [vA[4], kc0[0]], 6: [vA[5]], 7: [vA[6]]}
                for i in range(ST):
                    phase_a_tile(x_enc_t, xe_d, i)
                    for c in post[i]:
                        c()
                vA[7]()
                kc0[1]()
                k_s = k_s_next[0]
                v2 = v2_next[0]

            concat_t = []
            vch_cache = {}
            pending_pv = [None]

            def emit_pv(args):
                v2_, vc_, pt_ev_, pt_od_, concat_, blk_ = args
                ps_o = pstile()
                for kt_i in range(ST):
                    nc.tensor.matmul(
                        ps_o[0:64, 0:QB],
                        v2_[:, kt_i, vc_:vc_ + 64],
                        pt_ev_[:, kt_i, :],
                        start=(kt_i == 0), stop=(kt_i == ST - 1),
                        tile_position=(0, 0))
                    nc.tensor.matmul(
                        ps_o[64:128, 0:QB],
                        v2_[:, kt_i, vc_ + 64:vc_ + 128],
                        pt_od_[:, kt_i, :],
                        start=(kt_i == 0), stop=(kt_i == ST - 1),
                        tile_position=(0, 64))
                nc.vector.tensor_copy(
                    concat_[0:64, blk_ * QB:(blk_ + 1) * QB],
                    ps_o[0:64, 0:QB])
                nc.vector.tensor_copy(
                    concat_[64:128, blk_ * QB:(blk_ + 1) * QB],
                    ps_o[64:128, 0:QB])

            wo_pre = {}

            def load_wo(p):
                wo_r = []
                for half in range(2):
                    wo_sb = wop.tile([P, 512], f32, tag="wo")
                    nc.gpsimd.dma_start(
                        wo_sb[:],
                        wot_d[p * P:(p + 1) * P,
                              half * 512:(half + 1) * 512])
                    wr = worp.tile([P, 512], f16, tag="wor")
                    nc.vector.tensor_copy(wr[:], wo_sb[:])
                    wo_r.append(wr)
                wo_pre[p] = wo_r

            from contextlib import ExitStack as _ES
            for p in range(NP):
                _sc = nc.named_scope(f"pair{p}")
                _sc.__enter__()
                if p + 2 < NP:
                    load_wt(q_wt, wqt_d, p + 2, "qw")
                    load_wt(k_wt, wkt_d, p + 2, "kw")
                if p == NP - 1:
                    load_wo(0)
                    load_wo(1)
                chunks = []
                if p + 1 < NP:
                    chunks += q_chunks(p + 1)
                    chunks += k_chunks(p + 1)
                G = (p // 4 + 1) * 4
                if G < NP:
                    if G not in vch_cache:
                        vch_cache[G] = v_chunks(G)
                    chunks += [vch_cache[G][2 * (p % 4)],
                               vch_cache[G][2 * (p % 4) + 1]]
                vc = (p % 4) * P

                concat = ccp.tile([P, S], f16, tag="cc", name="concat")
                concat_t.append(concat)

                for blk in range(NB):
                    pt_ev = ptp.tile([P, ST, QB], f16, tag="pt", name="ptev")
                    pt_od = ptp.tile([P, ST, QB], f16, tag="pt", name="ptod")
                    for qtb in range(QTB):
                        qt = blk * QTB + qtb
                        qs = slice(qt * P, (qt + 1) * P)
                        ps_s = [pstileS(), pstileS()]
                        negmaxes = []
                        # h01 outer: one stationary load per head; reduce
                        # of head0 runs under head1's matmuls
                        for h01 in range(2):
                            for kk in range(2):
                                ks = slice(kk * 512, (kk + 1) * 512)
                                nc.tensor.matmul(
                                    ps_s[h01][:, ks],
                                    q_t[p][h01][:, qs], k_s[:, ks],
                                    start=True, stop=True)
                            negmax = stat()
                            nc.vector.tensor_reduce(
                                negmax[:], ps_s[h01][:], axis=AX,
                                op=OP.max, negate=True)
                            negmaxes.append(negmax)
                        if pending_pv[0] is not None:
                            emit_pv(pending_pv[0])
                            pending_pv[0] = None
                        elif chunks:
                            chunks.pop(0)()
                        for h01 in range(2):
                            pt_dst = pt_ev if h01 == 0 else pt_od
                            rsum, recip = stat(), stat()
                            p_e = pep.tile([P, S], f16, tag="pe")
                            nc.scalar.activation(
                                p_e[:], ps_s[h01][:], AF.Exp,
                                bias=negmaxes[h01][:], accum_out=rsum[:])
                            nc.vector.reciprocal(recip[:], rsum[:])
                            nc.vector.tensor_scalar_mul(
                                p_e[:], p_e[:], recip[:])
                            nc.sync.dma_start_transpose(
                                pt_dst[:, :, qtb * P:(qtb + 1) * P], p_e[:])
                    pending_pv[0] = (v2, vc, pt_ev, pt_od, concat, blk)
                # drain leftover prep chunks; the last block's PV carries
                # into the next pair's first qt slot
                for c in chunks:
                    c()
                if p + 1 < NP:
                    k_s = k_s_next[0]
                    if (p + 1) % 4 == 0:
                        v2 = v2_next[0]
                _sc.__exit__(None, None, None)
            emit_pv(pending_pv[0])

            # ---- phase D: out = concat @ W_out^T + b ----
            _scD = nc.named_scope("phaseD")
            _scD.__enter__()
            for sg in range(2):
                ps_big = [pstileS(), pstileS(), pstileS()]
                ps_sm = [pstile(), pstile()]

                def out_slot(sl, half):
                    # slots: 3 [128,1024] tiles (6 halves) + 2 [128,512]
                    idx = sl * 2 + half
                    if idx < 6:
                        return ps_big[idx // 2][:, (idx % 2) * 512:
                                                (idx % 2) * 512 + 512]
                    return ps_sm[idx - 6][:]

                for p in range(NP):
                    if p not in wo_pre:
                        load_wo(p)
                    wo_r = wo_pre.pop(p)
                    nxt = p + 1 if p + 1 < NP else (0 if sg == 0 else None)
                    if nxt is not None and nxt not in wo_pre:
                        load_wo(nxt)
                    for sl in range(4):
                        st = sg * 4 + sl
                        for half in range(2):
                            nc.tensor.matmul(
                                out_slot(sl, half),
                                concat_t[p][:, st * P:(st + 1) * P],
                                wo_r[half][:],
                                start=(p == 0), stop=(p == NP - 1))
                for sl in range(4):
                    st = sg * 4 + sl
                    out_sb = osbp.tile([P, H], f32, tag="osb")
                    for half in range(2):
                        nc.vector.tensor_tensor(
                            out_sb[:, half * 512:(half + 1) * 512],
                            out_slot(sl, half),
                            bias_sb[:, half * 512:(half + 1) * 512],
                            op=OP.add)
                    nc.scalar.dma_start(out_d[st * P:(st + 1) * P, :], out_sb[:])
            _scD.__exit__(None, None, None)

    nc.compile()
    return nc


def prep_in_maps(decoder_input, encoder_output, W_query, W_key, W_value,
                 W_out, b_out):
    f = lambda a: np.ascontiguousarray(np.asarray(a, dtype=np.float32))
    di = f(decoder_input)
    eo = f(encoder_output)
    wq = np.ascontiguousarray((f(W_query).reshape(H, H) * np.float32(0.125)).T)
    wk = np.ascontiguousarray(f(W_key).reshape(H, H).T)
    wv = np.ascontiguousarray(f(W_value).reshape(H, H).T)
    wo = np.ascontiguousarray(f(W_out).T)
    bias = np.ascontiguousarray(np.broadcast_to(f(b_out), (P, H)))
    return [
        {"xd": di[b], "xe": eo[b], "wqt": wq, "wkt": wk, "wvt": wv,
         "wot": wo, "bias": bias}
        for b in range(B)
    ]


_BUILT = None


def kernel(decoder_input, encoder_output, W_query, W_key, W_value, W_out,
           b_out):
    global _BUILT
    from concourse import bass_utils
    if _BUILT is None:
        _BUILT = build()
    in_maps = prep_in_maps(decoder_input, encoder_output, W_query, W_key,
                           W_value, W_out, b_out)
    try:
        res = bass_utils.run_bass_kernel_spmd(_BUILT, in_maps,
                                              core_ids=list(range(B)))
    except Exception:
        # one retry: a previously wedged NeuronCore can fail the first
        # execution after load
        res = bass_utils.run_bass_kernel_spmd(_BUILT, in_maps,
                                              core_ids=list(range(B)))
    return np.stack([res.results[b]["out"] for b in range(B)], axis=0)
